# revision 1
# baseline (speedup 1.0000x reference)
"""MAHN layer Trainium2 kernel: out[i] = w2[i] * sum_{e:(i,j)} w1[t_e] * relu(x@W)[j].

Strategy (8 NeuronCores, SPMD):
  - Destination-row partitioning: dests sorted by degree desc, round-robin to
    cores; each core owns 12500 dest rows organized as 98 tiles of 128.
  - Each core computes h = relu(x@W) for a contiguous 1/8 node slice, then
    AllGather -> full h table in local DRAM.
  - Per dest-tile, edges are packed into "planes": plane j holds the j-th
    edge of each of the tile's 128 dests (col index, or dummy with decay 0).
    One indirect DMA per plane gathers 128 h-rows (one per partition).
  - VectorE: multiply by per-edge decay (w1*w2 folded on host), then a
    strided tensor_reduce sums planes -> [128, 32] per tile.
"""
import numpy as np

N, E, DIN, DOUT = 100000, 1600000, 128, 32
NCORES = 8
PER = N // NCORES            # 12500 dests/core
TILES = (PER + 127) // 128   # 98
PERP = TILES * 128           # 12544 padded dests/core (also h-slice pad)


def _build(ptab):
    import concourse.bass as bass
    import concourse.tile as tile
    from concourse import bacc, mybir

    S = int(sum(ptab))
    nc = bacc.Bacc("TRN2", target_bir_lowering=False, debug=False,
                   num_devices=NCORES)
    f32, i32 = mybir.dt.float32, mybir.dt.int32

    xT = nc.dram_tensor("xT", [128, PER], f32, kind="ExternalInput").ap()
    W = nc.dram_tensor("W", [128, DOUT], f32, kind="ExternalInput").ap()
    idx = nc.dram_tensor("idx", [128, S], i32, kind="ExternalInput").ap()
    dec = nc.dram_tensor("dec", [128, S], f32, kind="ExternalInput").ap()
    out = nc.dram_tensor("out", [128, TILES * DOUT], f32,
                         kind="ExternalOutput").ap()

    with tile.TileContext(nc) as tc:
        with tc.tile_pool(name="sb", bufs=1) as sb, \
             tc.tile_pool(name="g", bufs=4) as gp, \
             tc.tile_pool(name="ps", bufs=4, space="PSUM") as ps, \
             tc.tile_pool(name="dram", bufs=1, space="DRAM") as dram:
            hslice = dram.tile([PERP, DOUT], f32)
            hfull = dram.tile([PERP * NCORES, DOUT], f32)

            xT_sb = sb.tile([128, PER], f32)
            W_sb = sb.tile([128, DOUT], f32)
            nc.sync.dma_start(xT_sb[:], xT[:])
            nc.sync.dma_start(W_sb[:], W[:])

            hst = sb.tile([128, TILES * DOUT], f32)
            for t in range(TILES):
                n0 = t * 128
                cols = min(128, PER - n0)
                hp = ps.tile([128, DOUT], f32, space="PSUM", tag="hp")
                nc.tensor.matmul(hp[:cols, :], lhsT=xT_sb[:, n0:n0 + cols],
                                 rhs=W_sb[:], start=True, stop=True)
                if cols < 128:
                    nc.vector.memset(hst[:, t * DOUT:(t + 1) * DOUT], 0.0)
                nc.scalar.activation(
                    out=hst[:cols, t * DOUT:(t + 1) * DOUT], in_=hp[:cols, :],
                    func=mybir.ActivationFunctionType.Relu)
            nc.sync.dma_start(
                hslice[:].rearrange("(t p) f -> p t f", p=128), hst[:])
            nc.gpsimd.collective_compute(
                "AllGather", mybir.AluOpType.bypass,
                replica_groups=[list(range(NCORES))],
                ins=[hslice.opt()], outs=[hfull.opt()])

            idx_sb = sb.tile([128, S], i32)
            dec_sb = sb.tile([128, S], f32)
            nc.sync.dma_start(idx_sb[:], idx[:])
            nc.sync.dma_start(dec_sb[:], dec[:])

            ost = sb.tile([128, TILES * DOUT], f32)
            off = 0
            for t in range(TILES):
                P = int(ptab[t])
                g = gp.tile([128, P * DOUT], f32, tag="g")
                for j in range(P):
                    nc.gpsimd.indirect_dma_start(
                        out=g[:, j * DOUT:(j + 1) * DOUT],
                        out_offset=None,
                        in_=hfull[:],
                        in_offset=bass.IndirectOffsetOnAxis(
                            ap=idx_sb[:, off + j:off + j + 1], axis=0),
                    )
                sc = gp.tile([128, P * DOUT], f32, tag="sc")
                nc.vector.tensor_tensor(
                    out=sc[:], in0=g[:],
                    in1=dec_sb[:, off:off + P, None].to_broadcast([128, P, DOUT]),
                    op=mybir.AluOpType.mult)
                nc.vector.tensor_reduce(
                    out=ost[:, t * DOUT:(t + 1) * DOUT],
                    in_=sc[:].rearrange("p (k f) -> p f k", f=DOUT),
                    axis=mybir.AxisListType.X, op=mybir.AluOpType.add)
                off += P
            nc.sync.dma_start(out[:], ost[:])
    nc.compile()
    return nc


def kernel(input, W, decay_weight1, decay_weight2, edge_row, edge_col,
           edge_time, arrive_time, observation_time):
    from concourse.bass_utils import run_bass_kernel_spmd

    input = np.asarray(input, dtype=np.float32)
    W = np.asarray(W, dtype=np.float32)
    w1 = np.asarray(decay_weight1, dtype=np.float32)[:, 0]
    w2 = np.asarray(decay_weight2, dtype=np.float32)[:, 0]
    edge_row = np.asarray(edge_row).astype(np.int64)
    edge_col = np.asarray(edge_col).astype(np.int64)
    edge_time = np.asarray(edge_time).astype(np.int64)
    arrive_time = np.asarray(arrive_time).astype(np.int64)
    obs = int(np.asarray(observation_time))

    # effective per-edge decay: w1[t_e] * w2[win(dest)]  (w2 folded per edge)
    win = (60 * obs - arrive_time - 1) % 3600
    dec_edge = (w1[edge_time] * w2[win[edge_row]]).astype(np.float32)

    # dest -> (core, slot): degree-sorted round-robin
    deg = np.bincount(edge_row, minlength=N)
    order = np.argsort(-deg, kind="stable")      # rank r -> dest id
    core_of = np.empty(N, np.int64)
    slot_of = np.empty(N, np.int64)
    core_of[order] = np.arange(N) % NCORES
    slot_of[order] = np.arange(N) // NCORES
    tile_of = slot_of // 128
    part_of = slot_of % 128

    # plane counts per tile (shared across cores): max degree in tile
    ptab = np.zeros(TILES, np.int64)
    np.maximum.at(ptab, tile_of, deg)
    ptab = np.maximum(ptab, 1)
    offs = np.concatenate([[0], np.cumsum(ptab)])
    S = int(offs[-1])

    # pack edges: per (core, tile, part), j-th edge -> column offs[tile]+j
    ec, er = edge_col, edge_row
    c = core_of[er]; t = tile_of[er]; p = part_of[er]
    ordk = np.lexsort((np.arange(E), p, t, c))
    cs, ts, ps, cols_s, dec_s = c[ordk], t[ordk], p[ordk], ec[ordk], dec_edge[ordk]
    key = (cs * TILES + ts) * 128 + ps
    first = np.r_[True, key[1:] != key[:-1]]
    grp_start = np.maximum.accumulate(np.where(first, np.arange(E), 0))
    j = np.arange(E) - grp_start

    # h-full row of node n: core n//PER at padded base
    hrow = (ec // PER) * PERP + (ec % PER)
    hrow_s = hrow[ordk]

    idx_all = np.zeros((NCORES, 128, S), np.int32)
    dec_all = np.zeros((NCORES, 128, S), np.float32)
    colpos = offs[ts] + j
    idx_all[cs, ps, colpos] = hrow_s
    dec_all[cs, ps, colpos] = dec_s

    inputT = np.ascontiguousarray(input.T)        # [128, N]

    nc = _build(ptab)
    in_maps = []
    for cc in range(NCORES):
        in_maps.append({
            "xT": np.ascontiguousarray(inputT[:, cc * PER:(cc + 1) * PER]),
            "W": W,
            "idx": idx_all[cc],
            "dec": dec_all[cc],
        })
    res = run_bass_kernel_spmd(nc, in_maps, list(range(NCORES)))

    out = np.zeros((N, DOUT), np.float32)
    tt = tile_of  # [N]
    pp = part_of
    for cc in range(NCORES):
        o = res.results[cc]["out"]               # [128, TILES*DOUT]
        mine = core_of == cc
        out[mine] = o.reshape(128, TILES, DOUT)[pp[mine], tt[mine]]
    return out

